# revision 36
# baseline (speedup 1.0000x reference)
"""Trainium2 Bass kernel for nn_Encoder (dense MLP with stochastic ternarization).

y = tanh(x @ (s1*T(w1,n1)) + b1) @ (s2*T(w2,n2)) + b2,  T(w,n) = (w-n>1) - (w-n<-1)

Sharding: tensor-parallel over the 16384 hidden dim across 8 cores. Each core
gets a 2048-wide hidden shard of w1/noise1/s1/b1 (column-sharded) and the
matching 2048-row shard of w2/noise2; x is replicated. Each core computes
partial yT = (h_shard @ w2_shard).T per 512-batch block; per (block, dout
quarter) ReduceScatters(add) hand core c the summed rows {256q+32c..+32}
where s2/b2 are applied. The host reassembles the full output.

Structure (v2):
- Ternary weights in fp8e4 ({-2,0,+2} exact). w1/n1 are host-packed into
  single 1 MiB (w,n) DMA blocks per (stripe, k-group) for near-peak HBM BW.
- Layer 1 runs mixed precision: k-tiles 0-11 as 6 fp8 DoubleRow matmuls
  (256-contraction each, x pre-quantized to fp8e4 on host), k-tiles 12-23
  as bf16 moving-operand matmuls. Offline numerics: rel err 0.0178 vs the
  2e-2 gate (all-bf16: 0.0053, all-fp8: 0.0241).
- Blocks 0-2's layer-1 m-groups interleave at supply-stripe granularity so
  PE consumption tracks the ternarize supply; then L2(0), L1(3), L2(1..3)
  stagger so the ReduceScatter chain overlaps compute.
- Each (block, dout-quarter) is a 256KB bf16 ReduceScatter issued as soon as
  its two dout tiles are stored, so only the last quarter trails the MM
  stream (~10us tail vs 82us with per-half RS at the end).
- x DMA is chunked per 4-k-tile group so the first matmul chain unblocks
  ~20us earlier.
"""

import sys

for _p in ("/opt/trn_rl_repo",):
    if _p not in sys.path:
        sys.path.insert(0, _p)

import numpy as np
import ml_dtypes

import concourse.bass as bass
import concourse.bacc as bacc
import concourse.mybir as mybir
import concourse.tile as tile
import concourse.bass_utils as _bass_utils
from concourse.bass_utils import run_bass_kernel_spmd

BF16 = mybir.dt.bfloat16
F32 = mybir.dt.float32
FP8 = mybir.dt.float8e4
NPBF16 = ml_dtypes.bfloat16
NPFP8 = ml_dtypes.float8_e4m3   # IEEE e4m3 == TRN FP8_EXP4 (max +-240)

N_CORES = 8
B = 2048
DIN = 3072
DHID = 16384
DOUT = 1024
HSH = DHID // N_CORES   # 2048
K1 = DIN // 128          # 24 contraction tiles, layer 1
KG1 = K1 // 4            # 6 groups of 4 k-tiles
NDR = 12                 # k-tiles 0..11 run fp8 DoubleRow (6 pairs)
K2 = HSH // 128          # 16 contraction tiles, layer 2
KG2 = K2 // 2            # 8 groups of 2 k2-tiles (1 MiB weight blocks)
NB = B // 512            # 4 batch blocks
MT = HSH // 128          # 16 hidden m-tiles
ND = DOUT // 128         # 8 dout tiles
QW = 512                 # ternarize quarter width
NQ = HSH // QW           # 4 quarters of the hidden shard
HROWS = DOUT // 2        # 512 dout rows per RS half
HCHUNK = HROWS // N_CORES  # 64 rows per core per half

BIGK = float(2 ** 30)

TANH = mybir.ActivationFunctionType.Tanh
MULT = mybir.AluOpType.mult
ADD = mybir.AluOpType.add
DR = mybir.MatmulPerfMode.DoubleRow


def build_bass():
    nc = bacc.Bacc("TRN2", target_bir_lowering=False, debug=False, num_devices=N_CORES)

    # x: fp8 pairs for k-tiles 0..11, bf16 for k-tiles 12..23
    xt8 = nc.dram_tensor("xt8", [NB, 128, NDR, 512], FP8, kind="ExternalInput")
    xt16 = nc.dram_tensor("xt16", [NB, 128, K1 - NDR, 512], BF16, kind="ExternalInput")
    # packed (w, n) blocks: [quarter, half-kgroup, part, {w,n}, 2, QW]
    wn1g = nc.dram_tensor("wn1g", [NQ, KG1 * 2, 128, 2, 2, QW], F32, kind="ExternalInput")
    s1h = nc.dram_tensor("s1h", [128, MT], F32, kind="ExternalInput")
    b1m = nc.dram_tensor("b1m", [128, MT], F32, kind="ExternalInput")
    wn2g = nc.dram_tensor("wn2g", [KG2, 128, 2, 2, DOUT], F32, kind="ExternalInput")
    # col 0: half-RS row map (blocks 0-2); col 1: quarter-RS row map (block 3)
    s2c = nc.dram_tensor("s2c", [128, 2], F32, kind="ExternalInput")
    b2c = nc.dram_tensor("b2c", [128, 2], F32, kind="ExternalInput")

    # rows 0:64  -> dout 64c .. 64c+64   (half 0)
    # rows 64:128-> dout 512+64c .. +64  (half 1)
    yTc = nc.dram_tensor("yTc", [128, B], F32, kind="ExternalOutput")

    with tile.TileContext(nc) as tc:
        with (
            tc.tile_pool(name="const", bufs=1) as cpool,
            tc.tile_pool(name="dram", bufs=1, space="DRAM") as dpool,
            tc.tile_pool(name="t2w1", bufs=KG1) as t2pool,
            tc.tile_pool(name="t2w2", bufs=1) as t22pool,
            tc.tile_pool(name="stage", bufs=3) as spool,
            tc.tile_pool(name="x8", bufs=3) as x8pool,
            tc.tile_pool(name="x16", bufs=3) as x16pool,
            tc.tile_pool(name="hblk", bufs=3 * MT) as hpool,
            tc.tile_pool(name="yblk", bufs=4) as ypool,
            tc.tile_pool(name="fin", bufs=1) as fpool,
            tc.tile_pool(name="ps1", bufs=6, space="PSUM") as pspool,
            tc.tile_pool(name="ps2", bufs=2, space="PSUM") as ps2pool,
        ):
            s1_sb = cpool.tile([128, MT], F32, tag="s1")
            b1_sb = cpool.tile([128, MT], F32, tag="b1")
            s2_sb = cpool.tile([128, 2], F32, tag="s2")
            b2_sb = cpool.tile([128, 2], F32, tag="b2")
            nc.scalar.dma_start(s1_sb[:], s1h[:, :])
            nc.scalar.dma_start(b1_sb[:], b1m[:, :])
            nc.scalar.dma_start(s2_sb[:], s2c[:, :])
            nc.scalar.dma_start(b2_sb[:], b2c[:, :])


            # per (block, dout-half) partial/scattered buffers (bf16)
            yT_nh = [[dpool.tile([HROWS, 512], BF16, tag=f"yTp{n}{h}",
                                 name=f"yT_n{n}h{h}") for h in range(2)]
                     for n in range(NB)]
            rs_nh = [[dpool.tile([HCHUNK, 512], BF16, tag=f"rs{n}{h}",
                                 name=f"rs_n{n}h{h}") for h in range(2)]
                     for n in range(NB)]
            # block 3 uses 4 quarter-RS ops so the last one trails by ~10us
            yT_3q = [dpool.tile([256, 512], BF16, tag=f"yT3q{q}",
                                name=f"yT_3q{q}") for q in range(4)]
            rs_3q = [dpool.tile([32, 512], BF16, tag=f"rs3q{q}",
                                name=f"rs_3q{q}") for q in range(4)]

            x8_tiles = {}
            x16_tiles = {}

            def load_x(b):
                x8_tiles[b] = x8pool.tile([128, NDR, 512], FP8, tag="x8",
                                          name=f"x8_{b}")
                x16_tiles[b] = x16pool.tile([128, K1 - NDR, 512], BF16, tag="x16",
                                            name=f"x16_{b}")
                for g in range(3):
                    nc.scalar.dma_start(x8_tiles[b][:, 4 * g:4 * g + 4, :],
                                        xt8[b][:, 4 * g:4 * g + 4, :])
                    nc.scalar.dma_start(x16_tiles[b][:, 4 * g:4 * g + 4, :],
                                        xt16[b][:, 4 * g:4 * g + 4, :])

            for b in (0, 1, 2):
                load_x(b)

            # ---- ternarize (packed 1 MiB (w,n) blocks) ----
            t2g = [t2pool.tile([128, 4, HSH], FP8, tag="t2", name=f"t2g_{kg}")
                   for kg in range(KG1)]
            t22 = t22pool.tile([128, K2, DOUT], FP8, tag="t22")

            _tern_q = [0]

            def tern_block(dst_ap, wn_src, sub_k, fd):
                # T = (q>1) - (q<-1) as two DVE indicator masks; no ACT pass.
                wn_t = spool.tile([128, 2, sub_k, fd], F32, tag="wn")
                # alternate DMA queues so weight-supply transfers pipeline
                qeng = nc.sync if _tern_q[0] % 2 == 0 else nc.gpsimd
                _tern_q[0] += 1
                qeng.dma_start(wn_t[:], wn_src)
                nc.vector.tensor_sub(wn_t[:, 0], wn_t[:, 0], wn_t[:, 1])
                nc.vector.tensor_single_scalar(
                    dst_ap, wn_t[:, 0], 1.0, mybir.AluOpType.is_gt)
                s_t = spool.tile([128, sub_k, fd], FP8, tag="sgn")
                nc.vector.tensor_single_scalar(
                    s_t[:], wn_t[:, 0], -1.0, mybir.AluOpType.is_lt)
                nc.vector.tensor_sub(dst_ap, dst_ap, s_t[:])

            # (col0, width) supply stripes; first two thin so the first
            # matmul chains unblock early
            SUPPLY = [(0, 128), (128, 128), (256, 256), (512, 512),
                      (1024, 512), (1536, 512)]
            for c0, cw in SUPPLY:
                q, qo = c0 // QW, c0 % QW
                for hk in range(KG1 * 2):
                    kg, hh = hk // 2, hk % 2
                    tern_block(
                        t2g[kg][:, 2 * hh:2 * hh + 2, c0:c0 + cw],
                        wn1g[q, hk][:, :, :, qo:qo + cw], 2, cw,
                    )
            for kg in range(KG2):
                for o in (0, 512):
                    tern_block(
                        t22[:, kg * 2:(kg + 1) * 2, o:o + 512],
                        wn2g[kg][:, :, :, o:o + 512], 2, 512,
                    )

            # ---- compute ----
            h_sets = {0: [], 1: [], 2: [], 3: []}

            def layer1_mtile(b, m):
                x8, x16 = x8_tiles[b], x16_tiles[b]
                ps = pspool.tile([128, 512], F32, tag="ps")
                for jp in range(NDR // 2):      # fp8 DoubleRow pairs
                    kg, p = jp // 2, jp % 2
                    nc.tensor.matmul(
                        ps[:],
                        t2g[kg][:, 2 * p:2 * p + 2, m * 128:(m + 1) * 128],
                        x8[:, 2 * jp:2 * jp + 2, :],
                        start=(jp == 0), stop=False, perf_mode=DR)
                for k in range(NDR, K1):        # bf16 k-tiles
                    nc.tensor.matmul(
                        ps[:],
                        t2g[k // 4][:, k % 4, m * 128:(m + 1) * 128],
                        x16[:, k - NDR, :],
                        start=False, stop=(k == K1 - 1))
                h_m = hpool.tile([128, 512], BF16, tag="h")
                nc.scalar.activation(
                    h_m[:], ps[:], TANH,
                    bias=b1_sb[:, m:m + 1], scale=s1_sb[:, m:m + 1],
                )
                h_sets[b].append(h_m)

            def rs_half(b, h):
                nc.gpsimd.collective_compute(
                    "ReduceScatter",
                    mybir.AluOpType.add,
                    replica_groups=[list(range(N_CORES))],
                    ins=[yT_nh[b][h].opt()],
                    outs=[rs_nh[b][h].opt()],
                )

            def rs_quarter3(q):
                nc.gpsimd.collective_compute(
                    "ReduceScatter",
                    mybir.AluOpType.add,
                    replica_groups=[list(range(N_CORES))],
                    ins=[yT_3q[q].opt()],
                    outs=[rs_3q[q].opt()],
                )

            def fin_block(b):
                # post-collective path on sync + DVE only
                rs_sb = fpool.tile([128, 512], BF16, tag="rsb")
                if b < 3:
                    nc.sync.dma_start(rs_sb[0:HCHUNK, :], rs_nh[b][0][:, :])
                    nc.sync.dma_start(rs_sb[HCHUNK:2 * HCHUNK, :],
                                      rs_nh[b][1][:, :])
                else:
                    for q in range(4):
                        nc.sync.dma_start(rs_sb[32 * q:32 * (q + 1), :],
                                          rs_3q[q][:, :])
                col = 0 if b < 3 else 1
                out_sb = fpool.tile([128, 512], F32, tag="osb")
                nc.vector.tensor_scalar(
                    out_sb[:], rs_sb[:], s2_sb[:, col:col + 1],
                    b2_sb[:, col:col + 1], MULT, ADD,
                )
                nc.sync.dma_start(yTc[:, b * 512:(b + 1) * 512], out_sb[:])

            def layer2_block(b):
                for d in range(ND):
                    p = ps2pool.tile([128, 512], F32, tag="ps2")
                    for k2 in range(K2):
                        nc.tensor.matmul(p[:], t22[:, k2, d * 128:(d + 1) * 128],
                                         h_sets[b][k2][:],
                                         start=(k2 == 0), stop=(k2 == K2 - 1))
                    y_sb = ypool.tile([128, 512], BF16, tag="y")
                    nc.vector.tensor_copy(y_sb[:], p[:])
                    if b < 3:
                        nc.scalar.dma_start(
                            yT_nh[b][d // 4][(d % 4) * 128:(d % 4 + 1) * 128, :],
                            y_sb[:],
                        )
                        if d == 3:
                            rs_half(b, 0)
                    else:
                        nc.scalar.dma_start(
                            yT_3q[d // 2][(d % 2) * 128:(d % 2 + 1) * 128, :],
                            y_sb[:],
                        )
                        if d % 2 == 1:
                            rs_quarter3(d // 2)
                if b < 3:
                    rs_half(b, 1)
                fin_block(b)

            # blocks 0-2 layer-1 interleaved m-major so consumption tracks
            # the column-ordered ternarize supply
            for m in range(MT):
                for b in (0, 1, 2):
                    layer1_mtile(b, m)

            # x3 queued ahead of L2(0)'s y-stores on the scalar queue
            load_x(3)
            layer2_block(0)
            for m in range(MT):
                layer1_mtile(3, m)
            layer2_block(1)
            layer2_block(2)
            layer2_block(3)

    nc.compile()
    return nc


_NC_CACHE = {}


def _get_nc():
    if "nc" not in _NC_CACHE:
        _NC_CACHE["nc"] = build_bass()
    return _NC_CACHE["nc"]


def _core_row_map(c):
    """Global yT rows owned by core c (half-RS map, blocks 0-2)."""
    rows = []
    for h in range(2):
        rows.extend(range(HROWS * h + HCHUNK * c, HROWS * h + HCHUNK * (c + 1)))
    return np.asarray(rows)


def _core_row_map_q(c):
    """Global yT rows owned by core c (quarter-RS map, block 3)."""
    rows = []
    for q in range(4):
        rows.extend(range(256 * q + 32 * c, 256 * q + 32 * (c + 1)))
    return np.asarray(rows)


def _make_in_maps(x, w1, s1, b1, w2, s2, b2, noise1, noise2):
    x = np.asarray(x, dtype=np.float32)
    w1 = np.asarray(w1, dtype=np.float32)
    s1 = np.asarray(s1, dtype=np.float32)
    b1 = np.asarray(b1, dtype=np.float32)
    w2 = np.asarray(w2, dtype=np.float32)
    s2 = np.asarray(s2, dtype=np.float32)
    b2 = np.asarray(b2, dtype=np.float32)
    noise1 = np.asarray(noise1, dtype=np.float32)
    noise2 = np.asarray(noise2, dtype=np.float32)

    xT = x.T
    # [DIN, B] -> [NB, 128, k, 512] for k-tile ranges
    def x_tile(rows, dt):
        nt = rows.stop - rows.start
        return np.ascontiguousarray(
            xT[rows].reshape(nt // 128, 128, NB, 512).transpose(2, 1, 0, 3)
        ).astype(dt)

    xt8 = x_tile(slice(0, NDR * 128), NPFP8)
    xt16 = x_tile(slice(NDR * 128, DIN), NPBF16)

    def w1_tile(w):   # [din, HSH] -> [NQ, KG1*2, 128, 2, QW]
        return w.reshape(KG1 * 2, 2, 128, NQ, QW).transpose(3, 0, 2, 1, 4)

    def w2_tile(w):   # [HSH, DOUT] -> [KG2, 128, 2, DOUT]
        return w.reshape(KG2, 2, 128, DOUT).transpose(0, 2, 1, 3)

    in_maps = []
    for c in range(N_CORES):
        hs = slice(c * HSH, (c + 1) * HSH)
        rows = _core_row_map(c)
        rows_q = _core_row_map_q(c)
        wn1 = np.ascontiguousarray(np.stack(
            [w1_tile(w1[:, hs]), w1_tile(noise1[:, hs])], axis=3))
        wn2 = np.ascontiguousarray(np.stack(
            [w2_tile(np.ascontiguousarray(w2[hs, :])),
             w2_tile(np.ascontiguousarray(noise2[hs, :]))], axis=2))
        in_maps.append({
            "xt8": xt8,
            "xt16": xt16,
            "wn1g": wn1,
            "s1h": np.ascontiguousarray(s1[hs].reshape(MT, 128).T),
            "b1m": np.ascontiguousarray(b1[hs].reshape(MT, 128).T),
            "wn2g": wn2,
            "s2c": np.ascontiguousarray(
                np.stack([s2[rows], s2[rows_q]], axis=1)),
            "b2c": np.ascontiguousarray(
                np.stack([b2[rows], b2[rows_q]], axis=1)),
        })
    return in_maps


def kernel(x, w1, s1, b1, w2, s2, b2, noise1, noise2, _bench_out=None):
    """Full-input, full-output entry point. Shards across 8 NeuronCores."""
    nc = _get_nc()
    in_maps = _make_in_maps(x, w1, s1, b1, w2, s2, b2, noise1, noise2)
    res = run_bass_kernel_spmd(nc, in_maps, core_ids=list(range(N_CORES)))
    if _bench_out is not None:
        _bench_out.append(res)
    yT = np.empty((DOUT, B), dtype=np.float32)
    for c in range(N_CORES):
        rows = _core_row_map(c)
        rows_q = _core_row_map_q(c)
        out_c = res.results[c]["yTc"]
        yT[rows, 0:1536] = out_c[:, 0:1536]
        yT[rows_q, 1536:2048] = out_c[:, 1536:2048]
    return np.ascontiguousarray(yT.T).astype(np.float32)


if __name__ == "__main__":
    nc = build_bass()
    print("built OK")
